# revision 9
# baseline (speedup 1.0000x reference)
"""AudioEncoder Trainium2 kernel, v2 (column-major conv scheme).

Computes: conv1d(1->64, k=5, stride=2, pad=2) + bias -> ReLU -> per-timestep
linear (64->64) + bias, over audio [4, 480000] f32 -> out [4, 240000, 64] f32.

Layout strategy (8 cores, pure data parallel; core = half of one batch row,
S = 120000 output positions):

  Host builds xcore [68, 3750] fp16 per core, xcore[p, c] = xpad[64c + p]
  (xpad = zero-padded audio, stride-2 conv => 64 input samples per 32 output
  positions).  Column c covers output positions 32c..32c+31; position
  i within a column needs taps xpad[64c + 2i .. 2i+4].

  Conv as matmul with 16 constant stationaries B_I [68, 128] fp16:
  B_I[p, gpar*64+ch] = wk[ch, p - 2*(I + 16*gpar)].  MM_I: psum[128, ncols]
  = B_I.T @ xcore[:, c0:c0+ncols] gives channels of positions (I, I+16) of
  every column -- plain contiguous moving operand, position permutation is
  free.  ACT applies conv bias + ReLU -> feats_j fp16 (j = I//2, two I per
  [128,1024] psum tile).

  Linear: per col-block of ml<=128 columns, 32 matmuls (g = I + 16*gpar):
  lhsT = feats_j[gpar*64:+64, u*512+m0:+ml] (stationary), rhs = w2 dup
  [gpar*64:+64, 0:64] -> psl[r, g*64+p] = out(pos 32*(c0+m0+r)+g, feat p).
  The gpar=0 / gpar=1 halves write DIFFERENT psum banks: the PE overlaps
  row-group-disjoint matmuls, and two concurrent matmuls writing the same
  partitions of one bank is a hardware fault (write-port conflict).  Within
  a bank all matmuls share one row-group, so they serialize safely.
  DVE adds (pre-broadcast) lin bias, casting f32 psum -> fp16 out tile.

  Store: outt[r, 0:2048] is exactly out[pos0+32r .. pos0+32r+31, 0:64] --
  one fully contiguous 4 KiB descriptor per partition, 512 KiB per store.
  Output is fp16 on device (quantization ~2.4e-4 << 2e-2 tol); host casts
  back to f32.

  PSUM: conv pool 2 bufs x 2 banks + linear pool 2 bufs x 2 banks = 8 banks.
"""

import numpy as np

import concourse.bacc as bacc
import concourse.bass as bass
import concourse.mybir as mybir
import concourse.tile as tile
from concourse.bass_utils import run_bass_kernel_spmd

B = 4
T = 480000
S_FULL = 240000
N_CORES = 8
S_CORE = S_FULL * B // N_CORES  # 120000
NC = S_CORE // 32  # 3750 columns per core
KP = 128  # xcore partitions (taps use rows 0-66; padded to 128 for DMA port
# coverage on loads and full-row stationaries)
E = 64
P = 64

f16 = mybir.dt.float16
f32 = mybir.dt.float32

SUPERS = [(j * 512, 512) for j in range(NC // 512)] + (
    [((NC // 512) * 512, NC % 512)] if NC % 512 else []
)


def col_blocks(ncols):
    out = []
    m0 = 0
    while m0 < ncols:
        out.append((m0, min(128, ncols - m0)))
        m0 += 128
    return out


def emit(nc: bass.Bass) -> None:
    from contextlib import ExitStack

    xp_d = nc.declare_dram_parameter("xp", [KP, NC], f16, isOutput=False)
    bi_d = nc.declare_dram_parameter("bi", [KP, 16 * 128], f16, isOutput=False)
    w2_d = nc.declare_dram_parameter("w2", [128, P], f16, isOutput=False)
    cb_d = nc.declare_dram_parameter("cb", [128, 1], f32, isOutput=False)
    b2_d = nc.declare_dram_parameter("b2", [128, 8 * P], f32, isOutput=False)
    out_d = nc.declare_dram_parameter("out", [S_CORE, P], f16, isOutput=True)

    RELU = mybir.ActivationFunctionType.Relu

    with tile.TileContext(nc) as tc, ExitStack() as ctx:
        consts = ctx.enter_context(tc.tile_pool(name="consts", bufs=1))
        fpool = ctx.enter_context(tc.tile_pool(name="feats", bufs=24))
        opool = ctx.enter_context(tc.tile_pool(name="outs", bufs=6))
        pc = ctx.enter_context(tc.tile_pool(name="psc", bufs=2, space="PSUM"))
        pl = ctx.enter_context(tc.tile_pool(name="psl", bufs=4, space="PSUM"))

        xp_sb = consts.tile([KP, NC], f16)
        bi_sb = consts.tile([KP, 16 * 128], f16)
        # first super's audio chunk + first conv weights first, so conv starts
        # early; the rest of bi and the remaining audio chunks follow
        c00, nc00 = SUPERS[0]
        nc.sync.dma_start(out=bi_sb[:, 0 : 4 * 128], in_=bi_d[:, 0 : 4 * 128])
        nc.sync.dma_start(out=xp_sb[:, c00 : c00 + nc00], in_=xp_d[:, c00 : c00 + nc00])
        cb_sb = consts.tile([128, 1], f32)
        nc.sync.dma_start(out=cb_sb[:, :], in_=cb_d[:, :])
        nc.sync.dma_start(out=bi_sb[:, 4 * 128 :], in_=bi_d[:, 4 * 128 :])
        for c0s, ncs in SUPERS[1:]:
            nc.sync.dma_start(out=xp_sb[:, c0s : c0s + ncs], in_=xp_d[:, c0s : c0s + ncs])
        w2_sb = consts.tile([128, P], f16)
        nc.sync.dma_start(out=w2_sb[:, :], in_=w2_d[:, :])
        b2_sb = consts.tile([128, 8 * P], f32)
        nc.sync.dma_start(out=b2_sb[:, :], in_=b2_d[:, :])

        # HAM warmup: ~3.5us of back-to-back matmuls into a throwaway psum
        # tile so the PE's activity monitor releases the clock throttle
        # (K=4/8 -> 8/8, 1.2 -> 2.4 GHz) before the real stream begins.
        # Rides the startup DMA window; gated only on the first bi/xp chunks.
        warm_ps = pl.tile([128, 512], f32, tag="psl")
        for _ in range(9):
            nc.tensor.matmul(
                out=warm_ps[:, 0:512],
                lhsT=bi_sb[:, 0:128],
                rhs=xp_sb[:, 0:512],
                start=True,
                stop=True,
            )

        def emit_conv_step(c0, ncols, j, featss):
            """Conv MMs for I = 2j, 2j+1 into one 2-bank psum tile + ACT."""
            psc = pc.tile([128, 1024], f32)
            for u in range(2):
                I = 2 * j + u
                nc.tensor.matmul(
                    out=psc[:, u * 512 : u * 512 + ncols],
                    lhsT=bi_sb[:, I * 128 : (I + 1) * 128],
                    rhs=xp_sb[:, c0 : c0 + ncols],
                    start=True,
                    stop=True,
                )
            feats = fpool.tile([128, 1024], f16)
            if ncols == 512:
                nc.scalar.activation(
                    out=feats[:, :], in_=psc[:, :], func=RELU,
                    bias=cb_sb[:, 0:1], scale=1.0,
                )
            else:
                for u in range(2):
                    nc.scalar.activation(
                        out=feats[:, u * 512 : u * 512 + ncols],
                        in_=psc[:, u * 512 : u * 512 + ncols],
                        func=RELU, bias=cb_sb[:, 0:1], scale=1.0,
                    )
            featss.append(feats)

        def emit_linear_half(prev, cb_i, h, outt):
            """16 linear MMs (positions-in-col g in {8h..8h+7} u {16+8h..+7}) ->
            two 1-bank psum tiles (A: feats rows 0-63 only, B: rows 64-127
            only -- same-bank MMs share a PE row-group so they serialize;
            A/B pairs overlap across banks), then DVE bias-add into outt."""
            featss, c0, ncols, mlist, _outts = prev
            m0, ml = mlist[cb_i]
            pslA = pl.tile([128, 512], f32, tag="psl")
            pslB = pl.tile([128, 512], f32, tag="psl")
            for gg in range(8):
                for gpar, psl in ((0, pslA), (1, pslB)):
                    I = 8 * h + gg
                    j, u = I // 2, I % 2
                    nc.tensor.matmul(
                        out=psl[0:ml, gg * 64 : (gg + 1) * 64],
                        lhsT=featss[j][
                            gpar * 64 : (gpar + 1) * 64,
                            u * 512 + m0 : u * 512 + m0 + ml,
                        ],
                        rhs=w2_sb[gpar * 64 : (gpar + 1) * 64, :],
                        start=True,
                        stop=True,
                    )
            nc.vector.tensor_add(
                outt[0:ml, h * 512 : (h + 1) * 512],
                pslA[0:ml, 0:512],
                b2_sb[0:ml, 0:512],
            )
            nc.vector.tensor_add(
                outt[0:ml, 1024 + h * 512 : 1024 + (h + 1) * 512],
                pslB[0:ml, 0:512],
                b2_sb[0:ml, 0:512],
            )

        def emit_linear_store(prev, cb_i, outt):
            featss, c0, ncols, mlist, _outts = prev
            m0, ml = mlist[cb_i]
            pos0 = 32 * (c0 + m0)
            dview = out_d[pos0 : pos0 + 32 * ml, :].rearrange(
                "(r g) p -> r (g p)", g=32
            )
            nc.sync.dma_start(out=dview, in_=outt[0:ml, 0 : 32 * P])

        prev = None
        for c0, ncols in SUPERS:
            featss = []
            for j in range(8):
                emit_conv_step(c0, ncols, j, featss)
                if prev is not None and (j // 2) < len(prev[3]):
                    cb_i, u2 = j // 2, j % 2
                    if u2 == 0:
                        prev_outt = opool.tile([128, 2048], f16, tag="outt")
                        prev[4].append(prev_outt)
                    emit_linear_half(prev, cb_i, u2, prev[4][-1])
                    if u2 == 1:
                        emit_linear_store(prev, cb_i, prev[4][-1])
            prev = (featss, c0, ncols, col_blocks(ncols), [])

        # drain: linear for the last super
        featss, c0, ncols, mlist, outts = prev
        for cb_i in range(len(mlist)):
            outt = opool.tile([128, 2048], f16, tag="outt")
            for u2 in range(2):
                emit_linear_half(prev, cb_i, u2, outt)
            emit_linear_store(prev, cb_i, outt)


def prep_shared(conv_w, conv_b, lin_w, lin_b):
    conv_w = np.asarray(conv_w, dtype=np.float32)
    conv_b = np.asarray(conv_b, dtype=np.float32)
    lin_w = np.asarray(lin_w, dtype=np.float32)
    lin_b = np.asarray(lin_b, dtype=np.float32)

    wk = conv_w[:, 0, :]  # [64, 5]
    bi = np.zeros((KP, 16 * 128), dtype=np.float16)
    for I in range(16):
        for gpar in range(2):
            i = I + 16 * gpar
            for k in range(5):
                p = 2 * i + k
                bi[p, I * 128 + gpar * 64 : I * 128 + (gpar + 1) * 64] = wk[
                    :, k
                ].astype(np.float16)
    w2 = lin_w.T.astype(np.float16)  # [64e, 64p]
    w2s = np.ascontiguousarray(np.concatenate([w2, w2], axis=0))  # [128, 64]
    cb = np.ascontiguousarray(
        np.concatenate([conv_b, conv_b]).astype(np.float32)[:, None]
    )  # [128, 1]
    b2 = np.ascontiguousarray(
        np.tile(lin_b.astype(np.float32)[None, :], (128, 8))
    )  # [128, 1024]
    return bi, w2s, cb, b2


def prep_inputs(audio_waveform, conv_w, conv_b, lin_w, lin_b):
    x = np.asarray(audio_waveform, dtype=np.float32)
    assert x.shape == (B, T)
    bi, w2s, cb, b2 = prep_shared(conv_w, conv_b, lin_w, lin_b)

    in_maps = []
    for c in range(N_CORES):
        b_i, h = divmod(c, 2)
        P0 = h * S_CORE
        xpad = np.zeros(2 * T + 2 * 64 + 8, dtype=np.float16)
        xpad[2 : 2 + T] = x[b_i].astype(np.float16)
        sw = np.lib.stride_tricks.sliding_window_view(xpad, KP)
        xcore = np.ascontiguousarray(
            sw[2 * P0 : 2 * P0 + 64 * NC : 64].T.astype(np.float16)
        )  # [68, 3750]
        in_maps.append(dict(xp=xcore, bi=bi, w2=w2s, cb=cb, b2=b2))
    return in_maps


_NC_CACHE = None


def get_nc() -> bass.Bass:
    global _NC_CACHE
    if _NC_CACHE is None:
        nc = bacc.Bacc()
        emit(nc)
        nc.compile()
        _NC_CACHE = nc
    return _NC_CACHE


def run(inputs: dict, trace: bool = False):
    in_maps = prep_inputs(**inputs)
    nc = get_nc()
    res = run_bass_kernel_spmd(nc, in_maps, list(range(N_CORES)), trace=trace)
    out = np.empty((B, S_FULL, P), dtype=np.float32)
    for c in range(N_CORES):
        b_i, h = divmod(c, 2)
        out[b_i, h * S_CORE : (h + 1) * S_CORE, :] = res.results[c]["out"].astype(
            np.float32
        )
    return out, res


def kernel(**inputs) -> np.ndarray:
    out, _ = run(inputs)
    return out


# revision 12
# speedup vs baseline: 1.1977x; 1.1977x over previous
"""AudioEncoder Trainium2 kernel, v2 (column-major conv scheme).

Computes: conv1d(1->64, k=5, stride=2, pad=2) + bias -> ReLU -> per-timestep
linear (64->64) + bias, over audio [4, 480000] f32 -> out [4, 240000, 64] f32.

Layout strategy (8 cores, pure data parallel; core = half of one batch row,
S = 120000 output positions):

  Host builds xcore [128, 3750] fp16 per core, xcore[p, c] = xpad[64c + p]
  (xpad = zero-padded audio, stride-2 conv => 64 input samples per 32 output
  positions).  Column c covers output positions 32c..32c+31; position
  i within a column needs taps xpad[64c + 2i .. 2i+4].

  Conv as matmul with 16 constant stationaries B_I [128, 128] fp16
  (tap rows 0-66 populated, rest zero):
  B_I[p, gpar*64+ch] = wk[ch, p - 2*(I + 16*gpar)].  MM_I: psum[128, ncols]
  = B_I.T @ xcore[:, c0:c0+ncols] gives channels of positions (I, I+16) of
  every column -- plain contiguous moving operand, position permutation is
  free.  ACT applies conv bias + ReLU -> feats_j fp16 (j = I//2, two I per
  [128,1024] psum tile).

  Linear: per col-block of ml<=128 columns, 32 matmuls (g = I + 16*gpar):
  lhsT = feats_j[gpar*64:+64, u*512+m0:+ml] (stationary), rhs = w2 dup
  [gpar*64:+64, 0:64] -> psl[r, g*64+p] = out(pos 32*(c0+m0+r)+g, feat p).
  The gpar=0 / gpar=1 halves write DIFFERENT psum banks: the PE overlaps
  row-group-disjoint matmuls, and two concurrent matmuls writing the same
  partitions of one bank is a hardware fault (write-port conflict).  Within
  a bank all matmuls share one row-group, so they serialize safely.
  DVE adds (pre-broadcast) lin bias, casting f32 psum -> fp16 out tile.

  Store: outt[r, 0:2048] is exactly out[pos0+32r .. pos0+32r+31, 0:64] --
  one fully contiguous 4 KiB descriptor per partition, 512 KiB per store.
  Output is fp16 on device (quantization ~2.4e-4 << 2e-2 tol); host casts
  back to f32.

  PSUM: conv pool 2 bufs x 2 banks + linear pool 2 bufs x 2 banks = 8 banks.
"""

import numpy as np

import concourse.bacc as bacc
import concourse.bass as bass
import concourse.mybir as mybir
import concourse.tile as tile
from concourse.bass_utils import run_bass_kernel_spmd

B = 4
T = 480000
S_FULL = 240000
N_CORES = 8
S_CORE = S_FULL * B // N_CORES  # 120000
NC = S_CORE // 32  # 3750 columns per core
KP = 128  # xcore partitions (taps use rows 0-66; padded to 128 for DMA port
# coverage on loads and full-row stationaries)
E = 64
P = 64

f16 = mybir.dt.float16
f32 = mybir.dt.float32

SUPERS = [(j * 512, 512) for j in range(NC // 512)] + (
    [((NC // 512) * 512, NC % 512)] if NC % 512 else []
)


def col_blocks(ncols):
    out = []
    m0 = 0
    while m0 < ncols:
        out.append((m0, min(128, ncols - m0)))
        m0 += 128
    return out


def emit(nc: bass.Bass) -> None:
    from contextlib import ExitStack

    xp_d = nc.declare_dram_parameter("xp", [KP, NC], f16, isOutput=False)
    bi_d = nc.declare_dram_parameter("bi", [KP, 16 * 128], f16, isOutput=False)
    w2_d = nc.declare_dram_parameter("w2", [128, P], f16, isOutput=False)
    cb_d = nc.declare_dram_parameter("cb", [128, 1], f32, isOutput=False)
    b2_d = nc.declare_dram_parameter("b2", [128, 8 * P], f32, isOutput=False)
    out_d = nc.declare_dram_parameter("out", [S_CORE, P], f16, isOutput=True)

    RELU = mybir.ActivationFunctionType.Relu

    with tile.TileContext(nc) as tc, ExitStack() as ctx:
        consts = ctx.enter_context(tc.tile_pool(name="consts", bufs=1))
        fpool = ctx.enter_context(tc.tile_pool(name="feats", bufs=24))
        opool = ctx.enter_context(tc.tile_pool(name="outs", bufs=6))
        pc = ctx.enter_context(tc.tile_pool(name="psc", bufs=2, space="PSUM"))
        pl = ctx.enter_context(tc.tile_pool(name="psl", bufs=4, space="PSUM"))

        xp_sb = consts.tile([KP, NC], f16)
        bi_sb = consts.tile([KP, 16 * 128], f16)
        # first super's audio chunk + first conv weights first, so conv starts
        # early; the rest of bi and the remaining audio chunks follow
        c00, nc00 = SUPERS[0]
        nc.sync.dma_start(out=bi_sb[:, 0 : 4 * 128], in_=bi_d[:, 0 : 4 * 128])
        nc.sync.dma_start(out=xp_sb[:, c00 : c00 + nc00], in_=xp_d[:, c00 : c00 + nc00])
        cb_sb = consts.tile([128, 1], f32)
        nc.sync.dma_start(out=cb_sb[:, :], in_=cb_d[:, :])
        nc.sync.dma_start(out=bi_sb[:, 4 * 128 :], in_=bi_d[:, 4 * 128 :])
        for c0s, ncs in SUPERS[1:]:
            nc.sync.dma_start(out=xp_sb[:, c0s : c0s + ncs], in_=xp_d[:, c0s : c0s + ncs])
        w2_sb = consts.tile([128, P], f16)
        nc.sync.dma_start(out=w2_sb[:, :], in_=w2_d[:, :])
        b2_sb = consts.tile([128, 8 * P], f32)
        nc.sync.dma_start(out=b2_sb[:, :], in_=b2_d[:, :])

        def emit_conv_step(c0, ncols, j, featss):
            """Conv MMs for I = 2j, 2j+1 into one 2-bank psum tile + ACT."""
            psc = pc.tile([128, 1024], f32)
            for u in range(2):
                I = 2 * j + u
                nc.tensor.matmul(
                    out=psc[:, u * 512 : u * 512 + ncols],
                    lhsT=bi_sb[:, I * 128 : (I + 1) * 128],
                    rhs=xp_sb[:, c0 : c0 + ncols],
                    start=True,
                    stop=True,
                )
            feats = fpool.tile([128, 1024], f16)
            if ncols == 512:
                nc.scalar.activation(
                    out=feats[:, :], in_=psc[:, :], func=RELU,
                    bias=cb_sb[:, 0:1], scale=1.0,
                )
            else:
                for u in range(2):
                    nc.scalar.activation(
                        out=feats[:, u * 512 : u * 512 + ncols],
                        in_=psc[:, u * 512 : u * 512 + ncols],
                        func=RELU, bias=cb_sb[:, 0:1], scale=1.0,
                    )
            featss.append(feats)

        def emit_linear_half(prev, cb_i, h, outt):
            """16 linear MMs (positions-in-col g in {8h..8h+7} u {16+8h..+7}) ->
            two 1-bank psum tiles (A: feats rows 0-63 only, B: rows 64-127
            only -- same-bank MMs share a PE row-group so they serialize;
            A/B pairs overlap across banks), then DVE bias-add into outt."""
            featss, c0, ncols, mlist, _outts = prev
            m0, ml = mlist[cb_i]
            pslA = pl.tile([128, 512], f32, tag="psl")
            pslB = pl.tile([128, 512], f32, tag="psl")
            for gg in range(8):
                for gpar, psl in ((0, pslA), (1, pslB)):
                    I = 8 * h + gg
                    j, u = I // 2, I % 2
                    nc.tensor.matmul(
                        out=psl[0:ml, gg * 64 : (gg + 1) * 64],
                        lhsT=featss[j][
                            gpar * 64 : (gpar + 1) * 64,
                            u * 512 + m0 : u * 512 + m0 + ml,
                        ],
                        rhs=w2_sb[gpar * 64 : (gpar + 1) * 64, :],
                        start=True,
                        stop=True,
                    )
            nc.vector.tensor_add(
                outt[0:ml, h * 512 : (h + 1) * 512],
                pslA[0:ml, 0:512],
                b2_sb[0:ml, 0:512],
            )
            nc.vector.tensor_add(
                outt[0:ml, 1024 + h * 512 : 1024 + (h + 1) * 512],
                pslB[0:ml, 0:512],
                b2_sb[0:ml, 0:512],
            )

        def emit_linear_store(prev, cb_i, outt):
            featss, c0, ncols, mlist, _outts = prev
            m0, ml = mlist[cb_i]
            pos0 = 32 * (c0 + m0)
            dview = out_d[pos0 : pos0 + 32 * ml, :].rearrange(
                "(r g) p -> r (g p)", g=32
            )
            nc.sync.dma_start(out=dview, in_=outt[0:ml, 0 : 32 * P])

        prev = None
        for c0, ncols in SUPERS:
            featss = []
            for j in range(8):
                emit_conv_step(c0, ncols, j, featss)
                if prev is not None and (j // 2) < len(prev[3]):
                    cb_i, u2 = j // 2, j % 2
                    if u2 == 0:
                        prev_outt = opool.tile([128, 2048], f16, tag="outt")
                        prev[4].append(prev_outt)
                    emit_linear_half(prev, cb_i, u2, prev[4][-1])
                    if u2 == 1:
                        emit_linear_store(prev, cb_i, prev[4][-1])
            prev = (featss, c0, ncols, col_blocks(ncols), [])

        # drain: linear for the last super
        featss, c0, ncols, mlist, outts = prev
        for cb_i in range(len(mlist)):
            outt = opool.tile([128, 2048], f16, tag="outt")
            for u2 in range(2):
                emit_linear_half(prev, cb_i, u2, outt)
            emit_linear_store(prev, cb_i, outt)


def prep_shared(conv_w, conv_b, lin_w, lin_b):
    conv_w = np.asarray(conv_w, dtype=np.float32)
    conv_b = np.asarray(conv_b, dtype=np.float32)
    lin_w = np.asarray(lin_w, dtype=np.float32)
    lin_b = np.asarray(lin_b, dtype=np.float32)

    wk = conv_w[:, 0, :]  # [64, 5]
    bi = np.zeros((KP, 16 * 128), dtype=np.float16)
    for I in range(16):
        for gpar in range(2):
            i = I + 16 * gpar
            for k in range(5):
                p = 2 * i + k
                bi[p, I * 128 + gpar * 64 : I * 128 + (gpar + 1) * 64] = wk[
                    :, k
                ].astype(np.float16)
    w2 = lin_w.T.astype(np.float16)  # [64e, 64p]
    w2s = np.ascontiguousarray(np.concatenate([w2, w2], axis=0))  # [128, 64]
    cb = np.ascontiguousarray(
        np.concatenate([conv_b, conv_b]).astype(np.float32)[:, None]
    )  # [128, 1]
    b2 = np.ascontiguousarray(
        np.tile(lin_b.astype(np.float32)[None, :], (128, 8))
    )  # [128, 1024]
    return bi, w2s, cb, b2


def prep_inputs(audio_waveform, conv_w, conv_b, lin_w, lin_b):
    x = np.asarray(audio_waveform, dtype=np.float32)
    assert x.shape == (B, T)
    bi, w2s, cb, b2 = prep_shared(conv_w, conv_b, lin_w, lin_b)

    in_maps = []
    for c in range(N_CORES):
        b_i, h = divmod(c, 2)
        P0 = h * S_CORE
        xpad = np.zeros(2 * T + 2 * 64 + 8, dtype=np.float16)
        xpad[2 : 2 + T] = x[b_i].astype(np.float16)
        sw = np.lib.stride_tricks.sliding_window_view(xpad, KP)
        xcore = np.ascontiguousarray(
            sw[2 * P0 : 2 * P0 + 64 * NC : 64].T.astype(np.float16)
        )  # [68, 3750]
        in_maps.append(dict(xp=xcore, bi=bi, w2=w2s, cb=cb, b2=b2))
    return in_maps


_NC_CACHE = None


def get_nc() -> bass.Bass:
    global _NC_CACHE
    if _NC_CACHE is None:
        nc = bacc.Bacc()
        emit(nc)
        nc.compile()
        _NC_CACHE = nc
    return _NC_CACHE


def run(inputs: dict, trace: bool = False):
    in_maps = prep_inputs(**inputs)
    nc = get_nc()
    res = run_bass_kernel_spmd(nc, in_maps, list(range(N_CORES)), trace=trace)
    out = np.empty((B, S_FULL, P), dtype=np.float32)
    for c in range(N_CORES):
        b_i, h = divmod(c, 2)
        out[b_i, h * S_CORE : (h + 1) * S_CORE, :] = res.results[c]["out"].astype(
            np.float32
        )
    return out, res


def kernel(**inputs) -> np.ndarray:
    out, _ = run(inputs)
    return out
